# revision 9
# baseline (speedup 1.0000x reference)
"""SSD MultiBox loss (loss_l/N, loss_c/N) on 8 Trainium2 NeuronCores.

Self-contained: hardcodes shapes B=128, P=8732, C=2, N=32, 8 cores.
Data-parallel over images: each core handles 16 images; host sums the
per-core partial sums and does the final division.
"""
import sys
sys.path.insert(0, "/opt/trn_rl_repo")
import numpy as np
import concourse.bacc as bacc
import concourse.bass as bass
import concourse.tile as tile
from concourse import mybir
from concourse.bass_utils import run_bass_kernel_spmd
from concourse.masks import make_identity

F32 = mybir.dt.float32
U8 = mybir.dt.uint8
AF = mybir.ActivationFunctionType
OP = mybir.AluOpType

B, P, C, NT = 128, 8732, 2, 32
NCORES = 8
BI = B // NCORES          # images per core = 16
NCH = 8                   # prior chunks per image
PC = 1092                 # cols per chunk (8*1092 = 8736 >= 8732)
PPAD = NCH * PC           # 8736
PVALID_LAST = P - 7 * PC  # 1088 valid cols in last chunk
TH = -1.0986123           # ln(1/3): score >= TH  <=>  iou >= 0.5
NEG_BIG = -1.0e30
N_BISECT = 16


def bcast_col(col_ap, n):
    """Broadcast a [P,1] column AP along free dim to [P,n] via stride 0."""
    return bass.AP(tensor=col_ap.tensor, offset=col_ap.offset,
                   ap=[col_ap.ap[0], [0, n]])


_CACHE = {}


def build():
    nc = bacc.Bacc("TRN2", target_bir_lowering=False, debug=False)

    loc_in = nc.dram_tensor("loc", [BI, P, 4], F32, kind="ExternalInput")
    conf_in = nc.dram_tensor("conf", [BI, P, C], F32, kind="ExternalInput")
    pri_in = nc.dram_tensor("priors", [P, 4], F32, kind="ExternalInput")
    tgt_in = nc.dram_tensor("targets", [BI, NT, 5], F32, kind="ExternalInput")
    out_t = nc.dram_tensor("out", [1, 8], F32, kind="ExternalOutput")
    import os
    DBG = os.environ.get("KDBG", "0") == "1"
    if DBG:
        dbg_best = nc.dram_tensor("dbg_best", [128, PC], F32, kind="ExternalOutput")
        dbg_s0 = nc.dram_tensor("dbg_s0", [128, PC], F32, kind="ExternalOutput")
        dbg_mcx = nc.dram_tensor("dbg_mcx", [128, PC], F32, kind="ExternalOutput")
        dbg_iw0 = nc.dram_tensor("dbg_iw0", [128, PC], F32, kind="ExternalOutput")

    with tile.TileContext(nc) as tc:
        import contextlib
        with contextlib.ExitStack() as ctx:
            persist = ctx.enter_context(tc.tile_pool(name="persist", bufs=1))
            hot = ctx.enter_context(tc.tile_pool(name="hot", bufs=2))
            work = ctx.enter_context(tc.tile_pool(name="work", bufs=1))
            small = ctx.enter_context(tc.tile_pool(name="small", bufs=1))
            psp = ctx.enter_context(tc.tile_pool(name="psum", bufs=2, space="PSUM"))

            # ---------------- prior attribute tiles [128, PC] ----------------
            # partition (i, c) = image i (0..15) x chunk c (0..7); col j -> prior c*PC + j
            CXP = persist.tile([128, PC], F32)
            CYP = persist.tile([128, PC], F32)
            WPT = persist.tile([128, PC], F32)
            HPT = persist.tile([128, PC], F32)
            for attr, dst in ((0, CXP), (1, CYP), (2, WPT), (3, HPT)):
                for c in range(NCH):
                    ncols = PC if c < 7 else PVALID_LAST
                    srcap = bass.AP(tensor=pri_in, offset=(c * PC * 4 + attr),
                                    ap=[[0, 16], [4, ncols]])
                    nc.sync.dma_start(out=dst[c::8, :ncols], in_=srcap)
            # pads: cols PVALID_LAST..PC on partitions (i,7) -- engine ops can't
            # address strided partitions, so DMA from inline constants instead
            npad = PC - PVALID_LAST
            padvals = np.stack([np.full(npad, 1.0e6, np.float32),
                                np.full(npad, 1.0, np.float32),
                                np.zeros(npad, np.float32)])
            padc = nc.inline_tensor(padvals, name="padconst")
            def pad_fill(dst, which):
                srcap = bass.AP(tensor=padc, offset=which * npad, ap=[[0, 16], [1, npad]])
                nc.sync.dma_start(out=dst[7::8, PVALID_LAST:PC], in_=srcap)
            for dst, which in ((CXP, 0), (CYP, 0), (WPT, 1), (HPT, 1)):
                pad_fill(dst, which)

            HWP = persist.tile([128, PC], F32)
            HHP = persist.tile([128, PC], F32)
            AREAP = persist.tile([128, PC], F32)
            nc.vector.tensor_scalar(HWP, WPT, 0.5, None, OP.mult)
            nc.vector.tensor_scalar(HHP, HPT, 0.5, None, OP.mult)
            XMINP = persist.tile([128, PC], F32)
            XMAXP = persist.tile([128, PC], F32)
            YMINP = persist.tile([128, PC], F32)
            YMAXP = persist.tile([128, PC], F32)
            nc.vector.tensor_tensor(XMINP, CXP, HWP, OP.subtract)
            nc.vector.tensor_tensor(XMAXP, CXP, HWP, OP.add)
            nc.vector.tensor_tensor(YMINP, CYP, HHP, OP.subtract)
            nc.vector.tensor_tensor(YMAXP, CYP, HHP, OP.add)
            nc.vector.tensor_tensor(AREAP, WPT, HPT, OP.mult)
            NL5WX = persist.tile([128, PC], F32, tag="CXP")
            nc.scalar.activation(NL5WX, WPT, AF.Ln)
            nc.vector.tensor_scalar(NL5WX, NL5WX, -5.0, None, OP.mult)
            NL5WY = persist.tile([128, PC], F32, tag="CYP")
            nc.scalar.activation(NL5WY, HPT, AF.Ln)
            nc.vector.tensor_scalar(NL5WY, NL5WY, -5.0, None, OP.mult)
            scratch = work.tile([128, PC], F32, tag="scratch")
            RWX10 = persist.tile([128, PC], F32, tag="HWP")
            nc.vector.reciprocal_approx_accurate(RWX10, WPT, scratch)
            nc.vector.tensor_scalar(RWX10, RWX10, 10.0, None, OP.mult)
            scratch2 = work.tile([128, PC], F32, tag="scratch")
            RWY10 = persist.tile([128, PC], F32, tag="HHP")
            nc.vector.reciprocal_approx_accurate(RWY10, HPT, scratch2)
            nc.vector.tensor_scalar(RWY10, RWY10, 10.0, None, OP.mult)

            # ---------------- truth tables [128, NT] ----------------
            traw = persist.tile([128, NT, 5], F32)
            src = bass.AP(tensor=tgt_in, offset=0,
                          ap=[[NT * 5, 16], [0, 8], [5, NT], [1, 5]])
            nc.sync.dma_start(out=traw, in_=src)
            X1T = traw[:, :, 0]
            Y1T = traw[:, :, 1]
            X2T = traw[:, :, 2]
            Y2T = traw[:, :, 3]

            DXT = persist.tile([128, NT], F32)   # x2-x1
            DYT = persist.tile([128, NT], F32)
            CMX = persist.tile([128, NT], F32)   # (x1+x2)/2
            CMY = persist.tile([128, NT], F32)
            HWT = persist.tile([128, NT], F32)   # (x2-x1)/2
            HHT = persist.tile([128, NT], F32)
            ART = persist.tile([128, NT], F32)   # truth area
            LDX = persist.tile([128, NT], F32)   # ln(x2-x1)
            LDY = persist.tile([128, NT], F32)
            nc.vector.tensor_tensor(DXT, X2T, X1T, OP.subtract)
            nc.vector.tensor_tensor(DYT, Y2T, Y1T, OP.subtract)
            nc.vector.tensor_tensor(CMX, X1T, X2T, OP.add)
            nc.vector.tensor_scalar(CMX, CMX, 0.5, None, OP.mult)
            nc.vector.tensor_tensor(CMY, Y1T, Y2T, OP.add)
            nc.vector.tensor_scalar(CMY, CMY, 0.5, None, OP.mult)
            nc.vector.tensor_scalar(HWT, DXT, 0.5, None, OP.mult)
            nc.vector.tensor_scalar(HHT, DYT, 0.5, None, OP.mult)
            nc.vector.tensor_tensor(ART, DXT, DYT, OP.mult)
            nc.scalar.activation(LDX, DXT, AF.Ln)
            nc.scalar.activation(LDY, DYT, AF.Ln)

            # ---------------- matching state ----------------
            BEST = persist.tile([128, PC], F32)
            MCX = persist.tile([128, PC], F32)
            MCY = persist.tile([128, PC], F32)
            MLX = persist.tile([128, PC], F32)
            MLY = persist.tile([128, PC], F32)
            nc.vector.memset(BEST, NEG_BIG)
            nc.vector.memset(MCX, 0.0)
            nc.vector.memset(MCY, 0.0)
            nc.vector.memset(MLX, 0.0)
            nc.vector.memset(MLY, 0.0)

            # ---------------- t-loop ----------------
            for t in range(NT):
                mnx = hot.tile([128, PC], F32, tag="ax")
                mxx = hot.tile([128, PC], F32, tag="ay")
                nc.vector.tensor_scalar(mnx, XMAXP, X2T[:, t:t + 1], None, OP.min)
                nc.vector.tensor_scalar(mxx, XMINP, X1T[:, t:t + 1], None, OP.max)
                iwr = hot.tile([128, PC], F32, tag="iwr")
                nc.vector.tensor_tensor(iwr, mnx, mxx, OP.subtract)
                mny = hot.tile([128, PC], F32, tag="ax")
                mxy = hot.tile([128, PC], F32, tag="ay")
                nc.vector.tensor_scalar(mny, YMAXP, Y2T[:, t:t + 1], None, OP.min)
                nc.vector.tensor_scalar(mxy, YMINP, Y1T[:, t:t + 1], None, OP.max)
                ihr = hot.tile([128, PC], F32, tag="ihr")
                nc.vector.tensor_tensor(ihr, mny, mxy, OP.subtract)
                iw = hot.tile([128, PC], F32, tag="ax")
                ih = hot.tile([128, PC], F32, tag="ay")
                nc.scalar.activation(iw, iwr, AF.Relu)
                nc.scalar.activation(ih, ihr, AF.Relu)
                prod = hot.tile([128, PC], F32, tag="iwr")
                nc.vector.tensor_tensor(prod, iw, ih, OP.mult)
                li = hot.tile([128, PC], F32, tag="ax")
                nc.scalar.activation(li, prod, AF.Ln)
                ls = hot.tile([128, PC], F32, tag="ay")
                nc.scalar.activation(ls, AREAP, AF.Ln, bias=ART[:, t:t + 1])
                score = hot.tile([128, PC], F32, tag="iwr")
                nc.vector.tensor_tensor(score, li, ls, OP.subtract)
                if DBG and t == 0:
                    nc.sync.dma_start(out=dbg_s0[:, :], in_=score)
                    nc.sync.dma_start(out=dbg_iw0[:, :], in_=prod)
                cmp = hot.tile([128, PC], U8, tag="cmp")
                nc.vector.tensor_tensor(cmp, score, BEST, OP.is_gt)
                nc.vector.copy_predicated(BEST, cmp, score)
                nc.vector.copy_predicated(MCX, cmp, bcast_col(CMX[:, t:t + 1], PC))
                nc.vector.copy_predicated(MCY, cmp, bcast_col(CMY[:, t:t + 1], PC))
                nc.vector.copy_predicated(MLX, cmp, bcast_col(LDX[:, t:t + 1], PC))
                nc.vector.copy_predicated(MLY, cmp, bcast_col(LDY[:, t:t + 1], PC))

            if DBG:
                nc.sync.dma_start(out=dbg_best[:, :], in_=BEST)
                nc.sync.dma_start(out=dbg_mcx[:, :], in_=MCX)
            # ---------------- pos mask & helpers ----------------
            POSF = persist.tile([128, PC], F32)
            nc.vector.tensor_scalar(POSF, BEST, TH, None, OP.is_ge)

            # ---------------- dense loc loss ----------------
            LOCD = persist.tile([128, PC, 4], F32)
            for c in range(NCH):
                ncols = PC if c < 7 else PVALID_LAST
                nc.sync.dma_start(out=LOCD[c::8, :ncols, :],
                                  in_=loc_in[:, c * PC:c * PC + ncols, :])

            sl1sum = persist.tile([128, PC], F32, tag="WPT")
            first = True
            for cc, (MT, PRC, RW, NL) in enumerate((
                    (MCX, (XMINP, XMAXP), RWX10, None), (MCY, (YMINP, YMAXP), RWY10, None),
                    (MLX, None, None, NL5WX), (MLY, None, None, NL5WY))):
                g = work.tile([128, PC], F32, tag="g")
                if PRC is not None:
                    ts_ = work.tile([128, PC], F32, tag="dd")
                    nc.vector.tensor_tensor(ts_, PRC[0], PRC[1], OP.add)
                    dd = work.tile([128, PC], F32, tag="g")
                    nc.vector.affine_then_add(dd, ts_, MT, scale=-0.5, bias=0.0)
                    g = work.tile([128, PC], F32, tag="dd")
                    nc.vector.tensor_tensor(g, dd, RW, OP.mult)
                else:
                    nc.vector.affine_then_add(g, MT, NL, scale=5.0, bias=0.0)
                d = work.tile([128, PC], F32, tag="dloc")
                nc.vector.tensor_tensor(d, LOCD[:, :, cc], g, OP.subtract)
                a = work.tile([128, PC], F32, tag="g")
                nc.scalar.activation(a, d, AF.Abs)
                amin = work.tile([128, PC], F32, tag="amin")
                nc.vector.tensor_scalar(amin, a, 1.0, None, OP.min)
                q = work.tile([128, PC], F32, tag="q")
                nc.scalar.activation(q, amin, AF.Square)
                t1 = work.tile([128, PC], F32, tag="dloc")
                nc.vector.tensor_tensor(t1, a, amin, OP.subtract)
                sl1 = work.tile([128, PC], F32, tag="dd")
                nc.vector.scalar_tensor_tensor(sl1, q, 0.5, t1, OP.mult, OP.add)
                if first:
                    nc.vector.tensor_copy(sl1sum, sl1)
                    first = False
                else:
                    nc.vector.tensor_tensor(sl1sum, sl1sum, sl1, OP.add)

            llcol = small.tile([128, 1], F32)
            trash = persist.tile([128, PC], F32, tag="HPT")
            nc.vector.affine_mul_reduce(trash, llcol, POSF, sl1sum, scale=1.0, bias=0.0)

            # ---------------- dense CE ----------------
            CONFD = persist.tile([128, PC, 2], F32)
            for c in range(NCH):
                ncols = PC if c < 7 else PVALID_LAST
                nc.sync.dma_start(out=CONFD[c::8, :ncols, :],
                                  in_=conf_in[:, c * PC:c * PC + ncols, :])

            DD = persist.tile([128, PC], F32)   # c1 - c0
            nc.vector.tensor_tensor(DD, CONFD[:, :, 1], CONFD[:, :, 0], OP.subtract)
            pad_fill(DD, 2)

            CE0 = persist.tile([128, PC], F32)
            aa = work.tile([128, PC], F32, tag="g")
            nc.scalar.activation(aa, DD, AF.Abs)
            ee = work.tile([128, PC], F32, tag="dd")
            nc.scalar.activation(ee, aa, AF.Exp, scale=-1.0)
            l1 = work.tile([128, PC], F32, tag="g")
            nc.scalar.activation(l1, ee, AF.Ln, bias=1.0)
            rr = work.tile([128, PC], F32, tag="dd")
            nc.scalar.activation(rr, DD, AF.Relu)
            nc.vector.tensor_tensor(CE0, rr, l1, OP.add)

            CE1 = persist.tile([128, PC], F32)   # softplus(-d) = ce0 - d
            nc.vector.tensor_tensor(CE1, CE0, DD, OP.subtract)

            V = persist.tile([128, PC], F32)     # mining vector (1-pos)*ce0
            vtrash = work.tile([128, PC], F32, tag="scratch")
            vacc = small.tile([128, 1], F32, tag="vacc")
            nc.vector.affine_mul_reduce(V, vacc, POSF, CE0, scale=-1.0, bias=1.0)
            pad_fill(V, 2)

            spce_col = small.tile([128, 1], F32)
            nc.vector.affine_mul_reduce(vtrash, spce_col, POSF, CE1, scale=1.0, bias=0.0)
            np_col = small.tile([128, 1], F32)
            nc.vector.affine_mul_reduce(trash, np_col, POSF, POSF, scale=1.0, bias=0.0)

            # ---------------- partition-group reduction helpers ----------------
            ident = small.tile([128, 128], F32)
            make_identity(nc, ident)
            mask16 = small.tile([128, 16], F32)     # [p, i] = (p//8 == i)
            io16 = small.tile([128, 16], mybir.dt.int32)
            nc.gpsimd.iota(io16, pattern=[[1, 16]], base=0, channel_multiplier=0)
            io16f = small.tile([128, 16], F32)
            nc.vector.tensor_copy(io16f, io16)
            grp_i = small.tile([128, 1], mybir.dt.int32)
            nc.gpsimd.iota(grp_i, pattern=[[0, 1]], base=0, channel_multiplier=1)
            grp_s = small.tile([128, 1], mybir.dt.int32)
            nc.vector.tensor_scalar(grp_s, grp_i, 3, None, OP.logical_shift_right)
            grp_sf = small.tile([128, 1], F32)
            nc.vector.tensor_copy(grp_sf, grp_s)
            nc.vector.tensor_scalar(mask16, io16f, grp_sf[:, 0:1], None, OP.is_equal)
            # mask16T [16, 128] via PE transpose
            psT = psp.tile([16, 128], F32, tag="psT")
            nc.tensor.transpose(psT, mask16, ident)
            mask16T = small.tile([16, 128], F32)
            nc.vector.tensor_copy(mask16T, psT)

            def reduce16(col, name):
                """[128,1] column -> [16,1] per-image sums."""
                ps = psp.tile([16, 1], F32, tag="red16")
                nc.tensor.matmul(ps, mask16, col, start=True, stop=True)
                out = small.tile([16, 1], F32, tag=name)
                nc.vector.tensor_copy(out, ps)
                return out

            def bcast128(x16, name):
                """[16,1] -> [128,1] per-image broadcast."""
                ps = psp.tile([128, 1], F32, tag="bc128")
                nc.tensor.matmul(ps, mask16T, x16, start=True, stop=True)
                out = small.tile([128, 1], F32, tag=name)
                nc.vector.tensor_copy(out, ps)
                return out

            np16 = reduce16(np_col, "np16")
            k16 = small.tile([16, 1], F32)
            nc.vector.tensor_scalar(k16, np16, 3.0, None, OP.mult)

            # ---------------- mining bisection ----------------
            lo16 = small.tile([16, 1], F32)
            hi16 = small.tile([16, 1], F32)
            nc.vector.memset(lo16, 0.0)
            nc.vector.memset(hi16, 16.0)
            for it in range(N_BISECT):
                dlt = small.tile([16, 1], F32, tag="dlt")
                nc.vector.tensor_tensor(dlt, hi16, lo16, OP.subtract)
                mid16 = small.tile([16, 1], F32, tag="mid16")
                nc.vector.scalar_tensor_tensor(mid16, dlt, 0.5, lo16, OP.mult, OP.add)
                tau = bcast128(mid16, "tau")
                gt = work.tile([128, PC], F32, tag="gt")
                nc.vector.tensor_scalar(gt, V, tau[:, 0:1], None, OP.is_gt)
                cntc = small.tile([128, 1], F32, tag="cntc")
                gtrash = work.tile([128, PC], F32, tag="gtrash")
                nc.vector.affine_mul_reduce(gtrash, cntc, gt, gt, scale=1.0, bias=0.0)
                cnt16 = reduce16(cntc, "cnt16")
                sel = small.tile([16, 1], U8, tag="sel")
                nc.vector.tensor_tensor(sel, cnt16, k16, OP.is_ge)
                nc.vector.copy_predicated(lo16, sel, mid16)
                seln = small.tile([16, 1], U8, tag="seln")
                nc.vector.tensor_tensor(seln, cnt16, k16, OP.is_lt)
                nc.vector.copy_predicated(hi16, seln, mid16)

            taus = bcast128(hi16, "taus")
            gt = work.tile([128, PC], F32, tag="gt")
            nc.vector.tensor_scalar(gt, V, taus[:, 0:1], None, OP.is_gt)
            sneg_col = small.tile([128, 1], F32)
            gtrash = work.tile([128, PC], F32, tag="gtrash")
            nc.vector.affine_mul_reduce(gtrash, sneg_col, gt, V, scale=1.0, bias=0.0)
            cnt_col = small.tile([128, 1], F32)
            gtrash2 = work.tile([128, PC], F32, tag="gtrash")
            nc.vector.affine_mul_reduce(gtrash2, cnt_col, gt, gt, scale=1.0, bias=0.0)

            sneg16 = reduce16(sneg_col, "sneg16")
            cnt16f = reduce16(cnt_col, "cnt16f")
            spce16 = reduce16(spce_col, "spce16")
            ll16 = reduce16(llcol, "ll16")

            # fix = (k - cnt) * tau ; loss_c per image = spce + sneg + fix
            fix16 = small.tile([16, 1], F32)
            nc.vector.tensor_tensor(fix16, k16, cnt16f, OP.subtract)
            nc.vector.tensor_tensor(fix16, fix16, hi16, OP.mult)
            lc16 = small.tile([16, 1], F32)
            nc.vector.tensor_tensor(lc16, spce16, sneg16, OP.add)
            nc.vector.tensor_tensor(lc16, lc16, fix16, OP.add)

            # final: stack [16, 7] and partition_all_reduce over 16
            fin = small.tile([16, 7], F32)
            nc.vector.tensor_copy(fin[:, 0:1], ll16)
            nc.vector.tensor_copy(fin[:, 1:2], lc16)
            nc.vector.tensor_copy(fin[:, 2:3], np16)
            nc.vector.tensor_copy(fin[:, 3:4], spce16)
            nc.vector.tensor_copy(fin[:, 4:5], sneg16)
            nc.vector.tensor_copy(fin[:, 5:6], fix16)
            nc.vector.tensor_copy(fin[:, 6:7], cnt16f)
            from concourse import bass_isa
            finr = small.tile([16, 7], F32)
            nc.gpsimd.partition_all_reduce(finr, fin, 16, bass_isa.ReduceOp.add)
            out_sb = small.tile([1, 8], F32)
            nc.vector.memset(out_sb, 0.0)
            nc.vector.tensor_copy(out_sb[0:1, 0:7], finr[0:1, :])
            nc.sync.dma_start(out=out_t[:, :], in_=out_sb)

    nc.compile()
    return nc


def kernel(loc_data, conf_data, priors, targets):
    if "nc" not in _CACHE:
        _CACHE["nc"] = build()
    nc = _CACHE["nc"]
    loc_data = np.ascontiguousarray(loc_data, dtype=np.float32)
    conf_data = np.ascontiguousarray(conf_data, dtype=np.float32)
    priors = np.ascontiguousarray(priors, dtype=np.float32)
    targets = np.ascontiguousarray(targets, dtype=np.float32)
    in_maps = []
    for c in range(NCORES):
        sl = slice(c * BI, (c + 1) * BI)
        in_maps.append(dict(loc=loc_data[sl], conf=conf_data[sl],
                            priors=priors, targets=targets[sl]))
    res = run_bass_kernel_spmd(nc, in_maps, list(range(NCORES)))
    ll = lc = npos = 0.0
    for r in res.results:
        o = r["out"][0]
        ll += float(o[0])
        lc += float(o[1])
        npos += float(o[2])
    n = np.float32(npos)
    return np.float32(ll) / n, np.float32(lc) / n


if __name__ == "__main__":
    import ref_np
    inp = ref_np.setup_inputs_np()
    out = kernel(**inp)
    print("kernel:", out)


# revision 11
# speedup vs baseline: 1.1388x; 1.1388x over previous
"""SSD MultiBox loss (loss_l/N, loss_c/N) on 8 Trainium2 NeuronCores.

Self-contained: hardcodes shapes B=128, P=8732, C=2, N=32, 8 cores.
Data-parallel over images: each core handles 16 images; host sums the
per-core partial sums and does the final division.
"""
import sys
sys.path.insert(0, "/opt/trn_rl_repo")
import numpy as np
import concourse.bacc as bacc
import concourse.bass as bass
import concourse.tile as tile
from concourse import mybir
from concourse.bass_utils import run_bass_kernel_spmd
from concourse.masks import make_identity

F32 = mybir.dt.float32
U8 = mybir.dt.uint8
AF = mybir.ActivationFunctionType
OP = mybir.AluOpType

B, P, C, NT = 128, 8732, 2, 32
NCORES = 8
BI = B // NCORES          # images per core = 16
NCH = 8                   # prior chunks per image
PC = 1092                 # cols per chunk (8*1092 = 8736 >= 8732)
PPAD = NCH * PC           # 8736
PVALID_LAST = P - 7 * PC  # 1088 valid cols in last chunk
TH = -1.0986123           # ln(1/3): score >= TH  <=>  iou >= 0.5
NEG_BIG = -1.0e30
N_BISECT = 12


def bcast_col(col_ap, n):
    """Broadcast a [P,1] column AP along free dim to [P,n] via stride 0."""
    return bass.AP(tensor=col_ap.tensor, offset=col_ap.offset,
                   ap=[col_ap.ap[0], [0, n]])


_CACHE = {}


def build():
    nc = bacc.Bacc("TRN2", target_bir_lowering=False, debug=False)

    loc_in = nc.dram_tensor("loc", [BI, P, 4], F32, kind="ExternalInput")
    conf_in = nc.dram_tensor("conf", [BI, P, C], F32, kind="ExternalInput")
    pri_in = nc.dram_tensor("priors", [P, 4], F32, kind="ExternalInput")
    tgt_in = nc.dram_tensor("targets", [BI, NT, 5], F32, kind="ExternalInput")
    out_t = nc.dram_tensor("out", [1, 8], F32, kind="ExternalOutput")
    import os
    DBG = os.environ.get("KDBG", "0") == "1"
    if DBG:
        dbg_best = nc.dram_tensor("dbg_best", [128, PC], F32, kind="ExternalOutput")
        dbg_s0 = nc.dram_tensor("dbg_s0", [128, PC], F32, kind="ExternalOutput")
        dbg_mcx = nc.dram_tensor("dbg_mcx", [128, PC], F32, kind="ExternalOutput")
        dbg_iw0 = nc.dram_tensor("dbg_iw0", [128, PC], F32, kind="ExternalOutput")

    with tile.TileContext(nc) as tc:
        import contextlib
        with contextlib.ExitStack() as ctx:
            persist = ctx.enter_context(tc.tile_pool(name="persist", bufs=1))
            hot = ctx.enter_context(tc.tile_pool(name="hot", bufs=3))
            work = ctx.enter_context(tc.tile_pool(name="work", bufs=1))
            small = ctx.enter_context(tc.tile_pool(name="small", bufs=1))
            psp = ctx.enter_context(tc.tile_pool(name="psum", bufs=2, space="PSUM"))

            # ---------------- prior attribute tiles [128, PC] ----------------
            # partition (i, c) = image i (0..15) x chunk c (0..7); col j -> prior c*PC + j
            CXP = persist.tile([128, PC], F32)
            CYP = persist.tile([128, PC], F32)
            WPT = persist.tile([128, PC], F32)
            HPT = persist.tile([128, PC], F32)
            for attr, dst in ((0, CXP), (1, CYP), (2, WPT), (3, HPT)):
                for c in range(NCH):
                    ncols = PC if c < 7 else PVALID_LAST
                    srcap = bass.AP(tensor=pri_in, offset=(c * PC * 4 + attr),
                                    ap=[[0, 16], [4, ncols]])
                    nc.sync.dma_start(out=dst[c::8, :ncols], in_=srcap)
            # pads: cols PVALID_LAST..PC on partitions (i,7) -- engine ops can't
            # address strided partitions, so DMA from inline constants instead
            npad = PC - PVALID_LAST
            padvals = np.stack([np.full(npad, 1.0e6, np.float32),
                                np.full(npad, 1.0, np.float32),
                                np.zeros(npad, np.float32)])
            padc = nc.inline_tensor(padvals, name="padconst")
            def pad_fill(dst, which):
                srcap = bass.AP(tensor=padc, offset=which * npad, ap=[[0, 16], [1, npad]])
                nc.sync.dma_start(out=dst[7::8, PVALID_LAST:PC], in_=srcap)
            for dst, which in ((CXP, 0), (CYP, 0), (WPT, 1), (HPT, 1)):
                pad_fill(dst, which)

            HWP = persist.tile([128, PC], F32)
            HHP = persist.tile([128, PC], F32)
            AREAP = persist.tile([128, PC], F32)
            nc.vector.tensor_scalar(HWP, WPT, 0.5, None, OP.mult)
            nc.vector.tensor_scalar(HHP, HPT, 0.5, None, OP.mult)
            XMINP = persist.tile([128, PC], F32)
            XMAXP = persist.tile([128, PC], F32)
            YMINP = persist.tile([128, PC], F32)
            YMAXP = persist.tile([128, PC], F32)
            nc.vector.tensor_tensor(XMINP, CXP, HWP, OP.subtract)
            nc.vector.tensor_tensor(XMAXP, CXP, HWP, OP.add)
            nc.vector.tensor_tensor(YMINP, CYP, HHP, OP.subtract)
            nc.vector.tensor_tensor(YMAXP, CYP, HHP, OP.add)
            nc.vector.tensor_tensor(AREAP, WPT, HPT, OP.mult)
            NL5WX = persist.tile([128, PC], F32, tag="CXP")
            nc.scalar.activation(NL5WX, WPT, AF.Ln)
            nc.vector.tensor_scalar(NL5WX, NL5WX, -5.0, None, OP.mult)
            NL5WY = persist.tile([128, PC], F32, tag="CYP")
            nc.scalar.activation(NL5WY, HPT, AF.Ln)
            nc.vector.tensor_scalar(NL5WY, NL5WY, -5.0, None, OP.mult)
            scratch = work.tile([128, PC], F32, tag="scratch")
            RWX10 = persist.tile([128, PC], F32, tag="HWP")
            nc.vector.reciprocal_approx_accurate(RWX10, WPT, scratch)
            nc.vector.tensor_scalar(RWX10, RWX10, 10.0, None, OP.mult)
            scratch2 = work.tile([128, PC], F32, tag="scratch")
            RWY10 = persist.tile([128, PC], F32, tag="HHP")
            nc.vector.reciprocal_approx_accurate(RWY10, HPT, scratch2)
            nc.vector.tensor_scalar(RWY10, RWY10, 10.0, None, OP.mult)

            # ---------------- truth tables [128, NT] ----------------
            traw = persist.tile([128, NT, 5], F32)
            src = bass.AP(tensor=tgt_in, offset=0,
                          ap=[[NT * 5, 16], [0, 8], [5, NT], [1, 5]])
            nc.sync.dma_start(out=traw, in_=src)
            X1T = traw[:, :, 0]
            Y1T = traw[:, :, 1]
            X2T = traw[:, :, 2]
            Y2T = traw[:, :, 3]

            DXT = persist.tile([128, NT], F32)   # x2-x1
            DYT = persist.tile([128, NT], F32)
            CMX = persist.tile([128, NT], F32)   # (x1+x2)/2
            CMY = persist.tile([128, NT], F32)
            HWT = persist.tile([128, NT], F32)   # (x2-x1)/2
            HHT = persist.tile([128, NT], F32)
            ART = persist.tile([128, NT], F32)   # truth area
            LDX = persist.tile([128, NT], F32)   # ln(x2-x1)
            LDY = persist.tile([128, NT], F32)
            nc.vector.tensor_tensor(DXT, X2T, X1T, OP.subtract)
            nc.vector.tensor_tensor(DYT, Y2T, Y1T, OP.subtract)
            nc.vector.tensor_tensor(CMX, X1T, X2T, OP.add)
            nc.vector.tensor_scalar(CMX, CMX, 0.5, None, OP.mult)
            nc.vector.tensor_tensor(CMY, Y1T, Y2T, OP.add)
            nc.vector.tensor_scalar(CMY, CMY, 0.5, None, OP.mult)
            nc.vector.tensor_scalar(HWT, DXT, 0.5, None, OP.mult)
            nc.vector.tensor_scalar(HHT, DYT, 0.5, None, OP.mult)
            nc.vector.tensor_tensor(ART, DXT, DYT, OP.mult)
            nc.scalar.activation(LDX, DXT, AF.Ln)
            nc.scalar.activation(LDY, DYT, AF.Ln)

            # ---------------- matching state ----------------
            BEST = persist.tile([128, PC], F32)
            MCX = persist.tile([128, PC], F32)
            MCY = persist.tile([128, PC], F32)
            MLX = persist.tile([128, PC], F32)
            MLY = persist.tile([128, PC], F32)
            nc.vector.memset(BEST, NEG_BIG)
            nc.vector.memset(MCX, 0.0)
            nc.vector.memset(MCY, 0.0)
            nc.vector.memset(MLX, 0.0)
            nc.vector.memset(MLY, 0.0)

            # ---------------- t-loop ----------------
            for t in range(NT):
                mxx = hot.tile([128, PC], F32, tag="ax")
                nc.vector.tensor_scalar(mxx, XMINP, X1T[:, t:t + 1], None, OP.max)
                iwr = hot.tile([128, PC], F32, tag="iwr")
                nc.vector.scalar_tensor_tensor(iwr, XMAXP, X2T[:, t:t + 1], mxx,
                                               OP.min, OP.subtract)
                mxy = hot.tile([128, PC], F32, tag="ay")
                nc.vector.tensor_scalar(mxy, YMINP, Y1T[:, t:t + 1], None, OP.max)
                ihr = hot.tile([128, PC], F32, tag="ihr")
                nc.vector.scalar_tensor_tensor(ihr, YMAXP, Y2T[:, t:t + 1], mxy,
                                               OP.min, OP.subtract)
                iw = hot.tile([128, PC], F32, tag="ax")
                ih = hot.tile([128, PC], F32, tag="ay")
                nc.scalar.activation(iw, iwr, AF.Relu)
                nc.scalar.activation(ih, ihr, AF.Relu)
                prod = hot.tile([128, PC], F32, tag="iwr")
                nc.vector.tensor_tensor(prod, iw, ih, OP.mult)
                li = hot.tile([128, PC], F32, tag="ax")
                nc.scalar.activation(li, prod, AF.Ln)
                ls = hot.tile([128, PC], F32, tag="ay")
                nc.scalar.activation(ls, AREAP, AF.Ln, bias=ART[:, t:t + 1])
                score = hot.tile([128, PC], F32, tag="iwr")
                nc.vector.tensor_tensor(score, li, ls, OP.subtract)
                if DBG and t == 0:
                    nc.sync.dma_start(out=dbg_s0[:, :], in_=score)
                    nc.sync.dma_start(out=dbg_iw0[:, :], in_=prod)
                cmp = work.tile([128, PC], U8, tag="cmp")
                nc.vector.tensor_tensor(cmp, score, BEST, OP.is_gt)
                nc.vector.copy_predicated(BEST, cmp, score)
                nc.vector.copy_predicated(MCX, cmp, bcast_col(CMX[:, t:t + 1], PC))
                nc.vector.copy_predicated(MCY, cmp, bcast_col(CMY[:, t:t + 1], PC))
                nc.vector.copy_predicated(MLX, cmp, bcast_col(LDX[:, t:t + 1], PC))
                nc.vector.copy_predicated(MLY, cmp, bcast_col(LDY[:, t:t + 1], PC))

            if DBG:
                nc.sync.dma_start(out=dbg_best[:, :], in_=BEST)
                nc.sync.dma_start(out=dbg_mcx[:, :], in_=MCX)
            # ---------------- pos mask & helpers ----------------
            POSF = persist.tile([128, PC], F32)
            nc.vector.tensor_scalar(POSF, BEST, TH, None, OP.is_ge)

            # ---------------- dense loc loss ----------------
            LOCD = persist.tile([128, PC, 4], F32)
            for c in range(NCH):
                ncols = PC if c < 7 else PVALID_LAST
                nc.sync.dma_start(out=LOCD[c::8, :ncols, :],
                                  in_=loc_in[:, c * PC:c * PC + ncols, :])

            sl1sum = persist.tile([128, PC], F32, tag="WPT")
            first = True
            for cc, (MT, PRC, RW, NL) in enumerate((
                    (MCX, (XMINP, XMAXP), RWX10, None), (MCY, (YMINP, YMAXP), RWY10, None),
                    (MLX, None, None, NL5WX), (MLY, None, None, NL5WY))):
                g = work.tile([128, PC], F32, tag="g")
                if PRC is not None:
                    ts_ = work.tile([128, PC], F32, tag="dd")
                    nc.vector.tensor_tensor(ts_, PRC[0], PRC[1], OP.add)
                    dd = work.tile([128, PC], F32, tag="g")
                    nc.vector.affine_then_add(dd, ts_, MT, scale=-0.5, bias=0.0)
                    g = work.tile([128, PC], F32, tag="dd")
                    nc.vector.tensor_tensor(g, dd, RW, OP.mult)
                else:
                    nc.vector.affine_then_add(g, MT, NL, scale=5.0, bias=0.0)
                d = work.tile([128, PC], F32, tag="dloc")
                nc.vector.tensor_tensor(d, LOCD[:, :, cc], g, OP.subtract)
                a = work.tile([128, PC], F32, tag="g")
                nc.scalar.activation(a, d, AF.Abs)
                amin = work.tile([128, PC], F32, tag="amin")
                nc.vector.tensor_scalar(amin, a, 1.0, None, OP.min)
                q = work.tile([128, PC], F32, tag="q")
                nc.scalar.activation(q, amin, AF.Square)
                t1 = work.tile([128, PC], F32, tag="dloc")
                nc.vector.tensor_tensor(t1, a, amin, OP.subtract)
                sl1 = work.tile([128, PC], F32, tag="dd")
                nc.vector.scalar_tensor_tensor(sl1, q, 0.5, t1, OP.mult, OP.add)
                if first:
                    nc.vector.tensor_copy(sl1sum, sl1)
                    first = False
                else:
                    nc.vector.tensor_tensor(sl1sum, sl1sum, sl1, OP.add)

            llcol = small.tile([128, 1], F32)
            trash = persist.tile([128, PC], F32, tag="HPT")
            nc.vector.affine_mul_reduce(trash, llcol, POSF, sl1sum, scale=1.0, bias=0.0)

            # ---------------- dense CE ----------------
            CONFD = persist.tile([128, PC, 2], F32)
            for c in range(NCH):
                ncols = PC if c < 7 else PVALID_LAST
                nc.sync.dma_start(out=CONFD[c::8, :ncols, :],
                                  in_=conf_in[:, c * PC:c * PC + ncols, :])

            DD = persist.tile([128, PC], F32)   # c1 - c0
            nc.vector.tensor_tensor(DD, CONFD[:, :, 1], CONFD[:, :, 0], OP.subtract)
            pad_fill(DD, 2)

            CE0 = persist.tile([128, PC], F32)
            aa = work.tile([128, PC], F32, tag="g")
            nc.scalar.activation(aa, DD, AF.Abs)
            ee = work.tile([128, PC], F32, tag="dd")
            nc.scalar.activation(ee, aa, AF.Exp, scale=-1.0)
            l1 = work.tile([128, PC], F32, tag="g")
            nc.scalar.activation(l1, ee, AF.Ln, bias=1.0)
            rr = work.tile([128, PC], F32, tag="dd")
            nc.scalar.activation(rr, DD, AF.Relu)
            nc.vector.tensor_tensor(CE0, rr, l1, OP.add)

            CE1 = persist.tile([128, PC], F32)   # softplus(-d) = ce0 - d
            nc.vector.tensor_tensor(CE1, CE0, DD, OP.subtract)

            V = persist.tile([128, PC], F32)     # mining vector (1-pos)*ce0
            vtrash = work.tile([128, PC], F32, tag="scratch")
            vacc = small.tile([128, 1], F32, tag="vacc")
            nc.vector.affine_mul_reduce(V, vacc, POSF, CE0, scale=-1.0, bias=1.0)
            pad_fill(V, 2)

            spce_col = small.tile([128, 1], F32)
            nc.vector.affine_mul_reduce(vtrash, spce_col, POSF, CE1, scale=1.0, bias=0.0)
            np_col = small.tile([128, 1], F32)
            nc.vector.affine_mul_reduce(trash, np_col, POSF, POSF, scale=1.0, bias=0.0)

            # ---------------- partition-group reduction helpers ----------------
            ident = small.tile([128, 128], F32)
            make_identity(nc, ident)
            mask16 = small.tile([128, 16], F32)     # [p, i] = (p//8 == i)
            io16 = small.tile([128, 16], mybir.dt.int32)
            nc.gpsimd.iota(io16, pattern=[[1, 16]], base=0, channel_multiplier=0)
            io16f = small.tile([128, 16], F32)
            nc.vector.tensor_copy(io16f, io16)
            grp_i = small.tile([128, 1], mybir.dt.int32)
            nc.gpsimd.iota(grp_i, pattern=[[0, 1]], base=0, channel_multiplier=1)
            grp_s = small.tile([128, 1], mybir.dt.int32)
            nc.vector.tensor_scalar(grp_s, grp_i, 3, None, OP.logical_shift_right)
            grp_sf = small.tile([128, 1], F32)
            nc.vector.tensor_copy(grp_sf, grp_s)
            nc.vector.tensor_scalar(mask16, io16f, grp_sf[:, 0:1], None, OP.is_equal)
            # mask16T [16, 128] via PE transpose
            psT = psp.tile([16, 128], F32, tag="psT")
            nc.tensor.transpose(psT, mask16, ident)
            mask16T = small.tile([16, 128], F32)
            nc.vector.tensor_copy(mask16T, psT)

            def reduce16(col, name):
                """[128,1] column -> [16,1] per-image sums."""
                ps = psp.tile([16, 1], F32, tag="red16")
                nc.tensor.matmul(ps, mask16, col, start=True, stop=True)
                out = small.tile([16, 1], F32, tag=name)
                nc.vector.tensor_copy(out, ps)
                return out

            def bcast128(x16, name):
                """[16,1] -> [128,1] per-image broadcast."""
                ps = psp.tile([128, 1], F32, tag="bc128")
                nc.tensor.matmul(ps, mask16T, x16, start=True, stop=True)
                out = small.tile([128, 1], F32, tag=name)
                nc.vector.tensor_copy(out, ps)
                return out

            np16 = reduce16(np_col, "np16")
            k16 = small.tile([16, 1], F32)
            nc.vector.tensor_scalar(k16, np16, 3.0, None, OP.mult)

            # ---------------- mining bisection ----------------
            lo16 = small.tile([16, 1], F32)
            hi16 = small.tile([16, 1], F32)
            nc.vector.memset(lo16, 0.0)
            nc.vector.memset(hi16, 16.0)
            for it in range(N_BISECT):
                dlt = small.tile([16, 1], F32, tag="dlt")
                nc.vector.tensor_tensor(dlt, hi16, lo16, OP.subtract)
                mid16 = small.tile([16, 1], F32, tag="mid16")
                nc.vector.scalar_tensor_tensor(mid16, dlt, 0.5, lo16, OP.mult, OP.add)
                tau = bcast128(mid16, "tau")
                gt = work.tile([128, PC], F32, tag="gt")
                nc.vector.tensor_scalar(gt, V, tau[:, 0:1], None, OP.is_gt)
                cntc = small.tile([128, 1], F32, tag="cntc")
                gtrash = work.tile([128, PC], F32, tag="gtrash")
                nc.vector.affine_mul_reduce(gtrash, cntc, gt, gt, scale=1.0, bias=0.0)
                cnt16 = reduce16(cntc, "cnt16")
                sel = small.tile([16, 1], U8, tag="sel")
                nc.vector.tensor_tensor(sel, cnt16, k16, OP.is_ge)
                nc.vector.copy_predicated(lo16, sel, mid16)
                seln = small.tile([16, 1], U8, tag="seln")
                nc.vector.tensor_tensor(seln, cnt16, k16, OP.is_lt)
                nc.vector.copy_predicated(hi16, seln, mid16)

            taus = bcast128(hi16, "taus")
            gt = work.tile([128, PC], F32, tag="gt")
            nc.vector.tensor_scalar(gt, V, taus[:, 0:1], None, OP.is_gt)
            sneg_col = small.tile([128, 1], F32)
            gtrash = work.tile([128, PC], F32, tag="gtrash")
            nc.vector.affine_mul_reduce(gtrash, sneg_col, gt, V, scale=1.0, bias=0.0)
            cnt_col = small.tile([128, 1], F32)
            gtrash2 = work.tile([128, PC], F32, tag="gtrash")
            nc.vector.affine_mul_reduce(gtrash2, cnt_col, gt, gt, scale=1.0, bias=0.0)

            sneg16 = reduce16(sneg_col, "sneg16")
            cnt16f = reduce16(cnt_col, "cnt16f")
            spce16 = reduce16(spce_col, "spce16")
            ll16 = reduce16(llcol, "ll16")

            # fix = (k - cnt) * tau ; loss_c per image = spce + sneg + fix
            fix16 = small.tile([16, 1], F32)
            nc.vector.tensor_tensor(fix16, k16, cnt16f, OP.subtract)
            nc.vector.tensor_tensor(fix16, fix16, hi16, OP.mult)
            lc16 = small.tile([16, 1], F32)
            nc.vector.tensor_tensor(lc16, spce16, sneg16, OP.add)
            nc.vector.tensor_tensor(lc16, lc16, fix16, OP.add)

            # final: stack [16, 7] and partition_all_reduce over 16
            fin = small.tile([16, 7], F32)
            nc.vector.tensor_copy(fin[:, 0:1], ll16)
            nc.vector.tensor_copy(fin[:, 1:2], lc16)
            nc.vector.tensor_copy(fin[:, 2:3], np16)
            nc.vector.tensor_copy(fin[:, 3:4], spce16)
            nc.vector.tensor_copy(fin[:, 4:5], sneg16)
            nc.vector.tensor_copy(fin[:, 5:6], fix16)
            nc.vector.tensor_copy(fin[:, 6:7], cnt16f)
            from concourse import bass_isa
            finr = small.tile([16, 7], F32)
            nc.gpsimd.partition_all_reduce(finr, fin, 16, bass_isa.ReduceOp.add)
            out_sb = small.tile([1, 8], F32)
            nc.vector.memset(out_sb, 0.0)
            nc.vector.tensor_copy(out_sb[0:1, 0:7], finr[0:1, :])
            nc.sync.dma_start(out=out_t[:, :], in_=out_sb)

    nc.compile()
    return nc


def kernel(loc_data, conf_data, priors, targets):
    if "nc" not in _CACHE:
        _CACHE["nc"] = build()
    nc = _CACHE["nc"]
    loc_data = np.ascontiguousarray(loc_data, dtype=np.float32)
    conf_data = np.ascontiguousarray(conf_data, dtype=np.float32)
    priors = np.ascontiguousarray(priors, dtype=np.float32)
    targets = np.ascontiguousarray(targets, dtype=np.float32)
    in_maps = []
    for c in range(NCORES):
        sl = slice(c * BI, (c + 1) * BI)
        in_maps.append(dict(loc=loc_data[sl], conf=conf_data[sl],
                            priors=priors, targets=targets[sl]))
    res = run_bass_kernel_spmd(nc, in_maps, list(range(NCORES)))
    ll = lc = npos = 0.0
    for r in res.results:
        o = r["out"][0]
        ll += float(o[0])
        lc += float(o[1])
        npos += float(o[2])
    n = np.float32(npos)
    return np.float32(ll) / n, np.float32(lc) / n


if __name__ == "__main__":
    import ref_np
    inp = ref_np.setup_inputs_np()
    out = kernel(**inp)
    print("kernel:", out)


# revision 12
# speedup vs baseline: 1.1475x; 1.0077x over previous
"""SSD MultiBox loss (loss_l/N, loss_c/N) on 8 Trainium2 NeuronCores.

Self-contained: hardcodes shapes B=128, P=8732, C=2, N=32, 8 cores.
Data-parallel over images: each core handles 16 images; host sums the
per-core partial sums and does the final division.
"""
import sys
sys.path.insert(0, "/opt/trn_rl_repo")
import numpy as np
import concourse.bacc as bacc
import concourse.bass as bass
import concourse.tile as tile
from concourse import mybir
from concourse.bass_utils import run_bass_kernel_spmd
from concourse.masks import make_identity

F32 = mybir.dt.float32
U8 = mybir.dt.uint8
AF = mybir.ActivationFunctionType
OP = mybir.AluOpType

B, P, C, NT = 128, 8732, 2, 32
NCORES = 8
BI = B // NCORES          # images per core = 16
NCH = 8                   # prior chunks per image
PC = 1092                 # cols per chunk (8*1092 = 8736 >= 8732)
PPAD = NCH * PC           # 8736
PVALID_LAST = P - 7 * PC  # 1088 valid cols in last chunk
TH = -1.0986123           # ln(1/3): score >= TH  <=>  iou >= 0.5
NEG_BIG = -1.0e30
N_BISECT = 12


def bcast_col(col_ap, n):
    """Broadcast a [P,1] column AP along free dim to [P,n] via stride 0."""
    return bass.AP(tensor=col_ap.tensor, offset=col_ap.offset,
                   ap=[col_ap.ap[0], [0, n]])


_CACHE = {}


def build():
    nc = bacc.Bacc("TRN2", target_bir_lowering=False, debug=False)

    loc_in = nc.dram_tensor("loc", [BI, P, 4], F32, kind="ExternalInput")
    conf_in = nc.dram_tensor("conf", [BI, P, C], F32, kind="ExternalInput")
    pri_in = nc.dram_tensor("priors", [P, 4], F32, kind="ExternalInput")
    tgt_in = nc.dram_tensor("targets", [BI, NT, 5], F32, kind="ExternalInput")
    out_t = nc.dram_tensor("out", [1, 8], F32, kind="ExternalOutput")
    import os
    DBG = os.environ.get("KDBG", "0") == "1"
    if DBG:
        dbg_best = nc.dram_tensor("dbg_best", [128, PC], F32, kind="ExternalOutput")
        dbg_s0 = nc.dram_tensor("dbg_s0", [128, PC], F32, kind="ExternalOutput")
        dbg_mcx = nc.dram_tensor("dbg_mcx", [128, PC], F32, kind="ExternalOutput")
        dbg_iw0 = nc.dram_tensor("dbg_iw0", [128, PC], F32, kind="ExternalOutput")

    with tile.TileContext(nc) as tc:
        import contextlib
        with contextlib.ExitStack() as ctx:
            persist = ctx.enter_context(tc.tile_pool(name="persist", bufs=1))
            hot = ctx.enter_context(tc.tile_pool(name="hot", bufs=3))
            work = ctx.enter_context(tc.tile_pool(name="work", bufs=1))
            small = ctx.enter_context(tc.tile_pool(name="small", bufs=1))
            psp = ctx.enter_context(tc.tile_pool(name="psum", bufs=2, space="PSUM"))

            # ---------------- prior attribute tiles [128, PC] ----------------
            # partition (i, c) = image i (0..15) x chunk c (0..7); col j -> prior c*PC + j
            CXP = persist.tile([128, PC], F32)
            CYP = persist.tile([128, PC], F32)
            WPT = persist.tile([128, PC], F32)
            HPT = persist.tile([128, PC], F32)
            for attr, dst in ((0, CXP), (1, CYP), (2, WPT), (3, HPT)):
                for c in range(NCH):
                    ncols = PC if c < 7 else PVALID_LAST
                    srcap = bass.AP(tensor=pri_in, offset=(c * PC * 4 + attr),
                                    ap=[[0, 16], [4, ncols]])
                    nc.sync.dma_start(out=dst[c::8, :ncols], in_=srcap)
            # pads: cols PVALID_LAST..PC on partitions (i,7) -- engine ops can't
            # address strided partitions, so DMA from inline constants instead
            npad = PC - PVALID_LAST
            padvals = np.stack([np.full(npad, 1.0e6, np.float32),
                                np.full(npad, 1.0, np.float32),
                                np.zeros(npad, np.float32)])
            padc = nc.inline_tensor(padvals, name="padconst")
            def pad_fill(dst, which):
                srcap = bass.AP(tensor=padc, offset=which * npad, ap=[[0, 16], [1, npad]])
                nc.sync.dma_start(out=dst[7::8, PVALID_LAST:PC], in_=srcap)
            for dst, which in ((CXP, 0), (CYP, 0), (WPT, 1), (HPT, 1)):
                pad_fill(dst, which)

            HWP = persist.tile([128, PC], F32)
            HHP = persist.tile([128, PC], F32)
            AREAP = persist.tile([128, PC], F32)
            nc.vector.tensor_scalar(HWP, WPT, 0.5, None, OP.mult)
            nc.vector.tensor_scalar(HHP, HPT, 0.5, None, OP.mult)
            XMINP = persist.tile([128, PC], F32)
            XMAXP = persist.tile([128, PC], F32)
            YMINP = persist.tile([128, PC], F32)
            YMAXP = persist.tile([128, PC], F32)
            nc.vector.tensor_tensor(XMINP, CXP, HWP, OP.subtract)
            nc.vector.tensor_tensor(XMAXP, CXP, HWP, OP.add)
            nc.vector.tensor_tensor(YMINP, CYP, HHP, OP.subtract)
            nc.vector.tensor_tensor(YMAXP, CYP, HHP, OP.add)
            nc.vector.tensor_tensor(AREAP, WPT, HPT, OP.mult)
            NL5WX = persist.tile([128, PC], F32, tag="CXP")
            nc.scalar.activation(NL5WX, WPT, AF.Ln)
            nc.vector.tensor_scalar(NL5WX, NL5WX, -5.0, None, OP.mult)
            NL5WY = persist.tile([128, PC], F32, tag="CYP")
            nc.scalar.activation(NL5WY, HPT, AF.Ln)
            nc.vector.tensor_scalar(NL5WY, NL5WY, -5.0, None, OP.mult)
            scratch = work.tile([128, PC], F32, tag="scratch")
            RWX10 = persist.tile([128, PC], F32, tag="HWP")
            nc.vector.reciprocal_approx_accurate(RWX10, WPT, scratch)
            nc.vector.tensor_scalar(RWX10, RWX10, 10.0, None, OP.mult)
            scratch2 = work.tile([128, PC], F32, tag="scratch")
            RWY10 = persist.tile([128, PC], F32, tag="HHP")
            nc.vector.reciprocal_approx_accurate(RWY10, HPT, scratch2)
            nc.vector.tensor_scalar(RWY10, RWY10, 10.0, None, OP.mult)

            # ---------------- truth tables [128, NT] ----------------
            traw = persist.tile([128, NT, 5], F32)
            src = bass.AP(tensor=tgt_in, offset=0,
                          ap=[[NT * 5, 16], [0, 8], [5, NT], [1, 5]])
            nc.sync.dma_start(out=traw, in_=src)
            X1T = traw[:, :, 0]
            Y1T = traw[:, :, 1]
            X2T = traw[:, :, 2]
            Y2T = traw[:, :, 3]

            DXT = persist.tile([128, NT], F32)   # x2-x1
            DYT = persist.tile([128, NT], F32)
            CMX = persist.tile([128, NT], F32)   # (x1+x2)/2
            CMY = persist.tile([128, NT], F32)
            HWT = persist.tile([128, NT], F32)   # (x2-x1)/2
            HHT = persist.tile([128, NT], F32)
            ART = persist.tile([128, NT], F32)   # truth area
            LDX = persist.tile([128, NT], F32)   # ln(x2-x1)
            LDY = persist.tile([128, NT], F32)
            nc.vector.tensor_tensor(DXT, X2T, X1T, OP.subtract)
            nc.vector.tensor_tensor(DYT, Y2T, Y1T, OP.subtract)
            nc.vector.tensor_tensor(CMX, X1T, X2T, OP.add)
            nc.vector.tensor_scalar(CMX, CMX, 0.5, None, OP.mult)
            nc.vector.tensor_tensor(CMY, Y1T, Y2T, OP.add)
            nc.vector.tensor_scalar(CMY, CMY, 0.5, None, OP.mult)
            nc.vector.tensor_scalar(HWT, DXT, 0.5, None, OP.mult)
            nc.vector.tensor_scalar(HHT, DYT, 0.5, None, OP.mult)
            nc.vector.tensor_tensor(ART, DXT, DYT, OP.mult)
            nc.scalar.activation(LDX, DXT, AF.Ln)
            nc.scalar.activation(LDY, DYT, AF.Ln)

            # ---------------- matching state ----------------
            BEST = persist.tile([128, PC], F32)
            MCX = persist.tile([128, PC], F32)
            MCY = persist.tile([128, PC], F32)
            MLX = persist.tile([128, PC], F32)
            MLY = persist.tile([128, PC], F32)
            nc.vector.memset(BEST, NEG_BIG)
            nc.vector.memset(MCX, 0.0)
            nc.vector.memset(MCY, 0.0)
            nc.vector.memset(MLX, 0.0)
            nc.vector.memset(MLY, 0.0)

            # ---------------- t-loop ----------------
            for t in range(NT):
                mxx = hot.tile([128, PC], F32, tag="ax")
                nc.vector.tensor_scalar(mxx, XMINP, X1T[:, t:t + 1], None, OP.max)
                iwr = hot.tile([128, PC], F32, tag="iwr")
                nc.vector.scalar_tensor_tensor(iwr, XMAXP, X2T[:, t:t + 1], mxx,
                                               OP.min, OP.subtract)
                mxy = hot.tile([128, PC], F32, tag="ay")
                nc.vector.tensor_scalar(mxy, YMINP, Y1T[:, t:t + 1], None, OP.max)
                ihr = hot.tile([128, PC], F32, tag="ihr")
                nc.vector.scalar_tensor_tensor(ihr, YMAXP, Y2T[:, t:t + 1], mxy,
                                               OP.min, OP.subtract)
                iw = hot.tile([128, PC], F32, tag="ax")
                ih = hot.tile([128, PC], F32, tag="ay")
                nc.scalar.activation(iw, iwr, AF.Relu)
                nc.scalar.activation(ih, ihr, AF.Relu)
                prod = hot.tile([128, PC], F32, tag="iwr")
                nc.vector.tensor_tensor(prod, iw, ih, OP.mult)
                li = hot.tile([128, PC], F32, tag="ax")
                nc.scalar.activation(li, prod, AF.Ln)
                ls = hot.tile([128, PC], F32, tag="ay")
                nc.scalar.activation(ls, AREAP, AF.Ln, bias=ART[:, t:t + 1])
                score = hot.tile([128, PC], F32, tag="iwr")
                nc.vector.tensor_tensor(score, li, ls, OP.subtract)
                if DBG and t == 0:
                    nc.sync.dma_start(out=dbg_s0[:, :], in_=score)
                    nc.sync.dma_start(out=dbg_iw0[:, :], in_=prod)
                cmp = work.tile([128, PC], U8, tag="cmp")
                nc.vector.tensor_tensor(cmp, score, BEST, OP.is_gt)
                nc.vector.copy_predicated(BEST, cmp, score)
                nc.vector.copy_predicated(MCX, cmp, bcast_col(CMX[:, t:t + 1], PC))
                nc.vector.copy_predicated(MCY, cmp, bcast_col(CMY[:, t:t + 1], PC))
                nc.vector.copy_predicated(MLX, cmp, bcast_col(LDX[:, t:t + 1], PC))
                nc.vector.copy_predicated(MLY, cmp, bcast_col(LDY[:, t:t + 1], PC))

            if DBG:
                nc.sync.dma_start(out=dbg_best[:, :], in_=BEST)
                nc.sync.dma_start(out=dbg_mcx[:, :], in_=MCX)
            # ---------------- pos mask & helpers ----------------
            POSF = persist.tile([128, PC], F32)
            nc.vector.tensor_scalar(POSF, BEST, TH, None, OP.is_ge)

            # ---------------- dense loc loss ----------------
            LOCD = persist.tile([128, PC, 4], F32)
            for c in range(NCH):
                ncols = PC if c < 7 else PVALID_LAST
                nc.sync.dma_start(out=LOCD[c::8, :ncols, :],
                                  in_=loc_in[:, c * PC:c * PC + ncols, :])

            sl1sum = persist.tile([128, PC], F32, tag="WPT")
            first = True
            for cc, (MT, PRC, RW, NL) in enumerate((
                    (MCX, (XMINP, XMAXP), RWX10, None), (MCY, (YMINP, YMAXP), RWY10, None),
                    (MLX, None, None, NL5WX), (MLY, None, None, NL5WY))):
                g = work.tile([128, PC], F32, tag="g")
                if PRC is not None:
                    ts_ = work.tile([128, PC], F32, tag="dd")
                    nc.vector.tensor_tensor(ts_, PRC[0], PRC[1], OP.add)
                    dd = work.tile([128, PC], F32, tag="g")
                    nc.vector.affine_then_add(dd, ts_, MT, scale=-0.5, bias=0.0)
                    g = work.tile([128, PC], F32, tag="dd")
                    nc.vector.tensor_tensor(g, dd, RW, OP.mult)
                else:
                    nc.vector.affine_then_add(g, MT, NL, scale=5.0, bias=0.0)
                d = work.tile([128, PC], F32, tag="dloc")
                nc.vector.tensor_tensor(d, LOCD[:, :, cc], g, OP.subtract)
                a = work.tile([128, PC], F32, tag="g")
                nc.scalar.activation(a, d, AF.Abs)
                amin = work.tile([128, PC], F32, tag="amin")
                nc.vector.tensor_scalar(amin, a, 1.0, None, OP.min)
                q = work.tile([128, PC], F32, tag="q")
                nc.scalar.activation(q, amin, AF.Square)
                t1 = work.tile([128, PC], F32, tag="dloc")
                nc.vector.tensor_tensor(t1, a, amin, OP.subtract)
                if first:
                    nc.vector.scalar_tensor_tensor(sl1sum, q, 0.5, t1, OP.mult, OP.add)
                    first = False
                else:
                    sl1 = work.tile([128, PC], F32, tag="dd")
                    nc.vector.scalar_tensor_tensor(sl1, q, 0.5, t1, OP.mult, OP.add)
                    nc.vector.tensor_tensor(sl1sum, sl1sum, sl1, OP.add)

            llcol = small.tile([128, 1], F32)
            trash = persist.tile([128, PC], F32, tag="HPT")
            nc.vector.affine_mul_reduce(trash, llcol, POSF, sl1sum, scale=1.0, bias=0.0)

            # ---------------- dense CE ----------------
            CONFD = persist.tile([128, PC, 2], F32)
            for c in range(NCH):
                ncols = PC if c < 7 else PVALID_LAST
                nc.sync.dma_start(out=CONFD[c::8, :ncols, :],
                                  in_=conf_in[:, c * PC:c * PC + ncols, :])

            DD = persist.tile([128, PC], F32)   # c1 - c0
            nc.vector.tensor_tensor(DD, CONFD[:, :, 1], CONFD[:, :, 0], OP.subtract)
            pad_fill(DD, 2)

            CE0 = persist.tile([128, PC], F32)
            aa = work.tile([128, PC], F32, tag="g")
            nc.scalar.activation(aa, DD, AF.Abs)
            ee = work.tile([128, PC], F32, tag="dd")
            nc.scalar.activation(ee, aa, AF.Exp, scale=-1.0)
            l1 = work.tile([128, PC], F32, tag="g")
            nc.scalar.activation(l1, ee, AF.Ln, bias=1.0)
            rr = work.tile([128, PC], F32, tag="dd")
            nc.scalar.activation(rr, DD, AF.Relu)
            nc.vector.tensor_tensor(CE0, rr, l1, OP.add)

            CE1 = persist.tile([128, PC], F32)   # softplus(-d) = ce0 - d
            nc.vector.tensor_tensor(CE1, CE0, DD, OP.subtract)

            V = persist.tile([128, PC], F32)     # mining vector (1-pos)*ce0
            vtrash = work.tile([128, PC], F32, tag="scratch")
            vacc = small.tile([128, 1], F32, tag="vacc")
            nc.vector.affine_mul_reduce(V, vacc, POSF, CE0, scale=-1.0, bias=1.0)
            pad_fill(V, 2)

            spce_col = small.tile([128, 1], F32)
            nc.vector.affine_mul_reduce(vtrash, spce_col, POSF, CE1, scale=1.0, bias=0.0)
            np_col = small.tile([128, 1], F32)
            nc.scalar.activation(trash, POSF, AF.Identity, accum_out=np_col)

            # ---------------- partition-group reduction helpers ----------------
            ident = small.tile([128, 128], F32)
            make_identity(nc, ident)
            mask16 = small.tile([128, 16], F32)     # [p, i] = (p//8 == i)
            io16 = small.tile([128, 16], mybir.dt.int32)
            nc.gpsimd.iota(io16, pattern=[[1, 16]], base=0, channel_multiplier=0)
            io16f = small.tile([128, 16], F32)
            nc.vector.tensor_copy(io16f, io16)
            grp_i = small.tile([128, 1], mybir.dt.int32)
            nc.gpsimd.iota(grp_i, pattern=[[0, 1]], base=0, channel_multiplier=1)
            grp_s = small.tile([128, 1], mybir.dt.int32)
            nc.vector.tensor_scalar(grp_s, grp_i, 3, None, OP.logical_shift_right)
            grp_sf = small.tile([128, 1], F32)
            nc.vector.tensor_copy(grp_sf, grp_s)
            nc.vector.tensor_scalar(mask16, io16f, grp_sf[:, 0:1], None, OP.is_equal)
            # mask16T [16, 128] via PE transpose
            psT = psp.tile([16, 128], F32, tag="psT")
            nc.tensor.transpose(psT, mask16, ident)
            mask16T = small.tile([16, 128], F32)
            nc.vector.tensor_copy(mask16T, psT)

            def reduce16(col, name):
                """[128,1] column -> [16,1] per-image sums."""
                ps = psp.tile([16, 1], F32, tag="red16")
                nc.tensor.matmul(ps, mask16, col, start=True, stop=True)
                out = small.tile([16, 1], F32, tag=name)
                nc.vector.tensor_copy(out, ps)
                return out

            def bcast128(x16, name):
                """[16,1] -> [128,1] per-image broadcast."""
                ps = psp.tile([128, 1], F32, tag="bc128")
                nc.tensor.matmul(ps, mask16T, x16, start=True, stop=True)
                out = small.tile([128, 1], F32, tag=name)
                nc.vector.tensor_copy(out, ps)
                return out

            np16 = reduce16(np_col, "np16")
            k16 = small.tile([16, 1], F32)
            nc.vector.tensor_scalar(k16, np16, 3.0, None, OP.mult)
            # sign-count threshold: cnt_img >= k  <=>  ssum_img >= 2k - 8736
            k216 = small.tile([16, 1], F32)
            nc.vector.tensor_scalar(k216, k16, 2.0, -8736.0, OP.mult, OP.add)

            # ---------------- mining bisection ----------------
            lo16 = small.tile([16, 1], F32)
            hi16 = small.tile([16, 1], F32)
            nc.vector.memset(lo16, 0.0)
            nc.vector.memset(hi16, 16.0)
            for it in range(N_BISECT):
                dlt = small.tile([16, 1], F32, tag="dlt")
                nc.vector.tensor_tensor(dlt, hi16, lo16, OP.subtract)
                mid16 = small.tile([16, 1], F32, tag="mid16")
                nc.vector.scalar_tensor_tensor(mid16, dlt, 0.5, lo16, OP.mult, OP.add)
                nmid16 = small.tile([16, 1], F32, tag="nmid16")
                nc.vector.tensor_scalar(nmid16, mid16, -1.0, None, OP.mult)
                ntau = bcast128(nmid16, "tau")
                sgn = work.tile([128, PC], F32, tag="gt")
                cntc = small.tile([128, 1], F32, tag="cntc")
                nc.scalar.activation(sgn, V, AF.Sign, bias=ntau[:, 0:1],
                                     accum_out=cntc)
                cnt16 = reduce16(cntc, "cnt16")
                sel = small.tile([16, 1], U8, tag="sel")
                nc.vector.tensor_tensor(sel, cnt16, k216, OP.is_ge)
                nc.vector.copy_predicated(lo16, sel, mid16)
                seln = small.tile([16, 1], U8, tag="seln")
                nc.vector.tensor_tensor(seln, cnt16, k216, OP.is_lt)
                nc.vector.copy_predicated(hi16, seln, mid16)

            taus = bcast128(hi16, "taus")
            gt = work.tile([128, PC], F32, tag="gt")
            nc.vector.tensor_scalar(gt, V, taus[:, 0:1], None, OP.is_gt)
            sneg_col = small.tile([128, 1], F32)
            gtrash = work.tile([128, PC], F32, tag="gtrash")
            nc.vector.affine_mul_reduce(gtrash, sneg_col, gt, V, scale=1.0, bias=0.0)
            cnt_col = small.tile([128, 1], F32)
            gtrash2 = work.tile([128, PC], F32, tag="gtrash")
            nc.vector.affine_mul_reduce(gtrash2, cnt_col, gt, gt, scale=1.0, bias=0.0)

            sneg16 = reduce16(sneg_col, "sneg16")
            cnt16f = reduce16(cnt_col, "cnt16f")
            spce16 = reduce16(spce_col, "spce16")
            ll16 = reduce16(llcol, "ll16")

            # fix = (k - cnt) * tau ; loss_c per image = spce + sneg + fix
            fix16 = small.tile([16, 1], F32)
            nc.vector.tensor_tensor(fix16, k16, cnt16f, OP.subtract)
            nc.vector.tensor_tensor(fix16, fix16, hi16, OP.mult)
            lc16 = small.tile([16, 1], F32)
            nc.vector.tensor_tensor(lc16, spce16, sneg16, OP.add)
            nc.vector.tensor_tensor(lc16, lc16, fix16, OP.add)

            # final: stack [16, 7] and partition_all_reduce over 16
            fin = small.tile([16, 7], F32)
            nc.vector.tensor_copy(fin[:, 0:1], ll16)
            nc.vector.tensor_copy(fin[:, 1:2], lc16)
            nc.vector.tensor_copy(fin[:, 2:3], np16)
            nc.vector.tensor_copy(fin[:, 3:4], spce16)
            nc.vector.tensor_copy(fin[:, 4:5], sneg16)
            nc.vector.tensor_copy(fin[:, 5:6], fix16)
            nc.vector.tensor_copy(fin[:, 6:7], cnt16f)
            from concourse import bass_isa
            finr = small.tile([16, 7], F32)
            nc.gpsimd.partition_all_reduce(finr, fin, 16, bass_isa.ReduceOp.add)
            out_sb = small.tile([1, 8], F32)
            nc.vector.memset(out_sb, 0.0)
            nc.vector.tensor_copy(out_sb[0:1, 0:7], finr[0:1, :])
            nc.sync.dma_start(out=out_t[:, :], in_=out_sb)

    nc.compile()
    return nc


def kernel(loc_data, conf_data, priors, targets):
    if "nc" not in _CACHE:
        _CACHE["nc"] = build()
    nc = _CACHE["nc"]
    loc_data = np.ascontiguousarray(loc_data, dtype=np.float32)
    conf_data = np.ascontiguousarray(conf_data, dtype=np.float32)
    priors = np.ascontiguousarray(priors, dtype=np.float32)
    targets = np.ascontiguousarray(targets, dtype=np.float32)
    in_maps = []
    for c in range(NCORES):
        sl = slice(c * BI, (c + 1) * BI)
        in_maps.append(dict(loc=loc_data[sl], conf=conf_data[sl],
                            priors=priors, targets=targets[sl]))
    res = run_bass_kernel_spmd(nc, in_maps, list(range(NCORES)))
    ll = lc = npos = 0.0
    for r in res.results:
        o = r["out"][0]
        ll += float(o[0])
        lc += float(o[1])
        npos += float(o[2])
    n = np.float32(npos)
    return np.float32(ll) / n, np.float32(lc) / n


if __name__ == "__main__":
    import ref_np
    inp = ref_np.setup_inputs_np()
    out = kernel(**inp)
    print("kernel:", out)
